# revision 1
# baseline (speedup 1.0000x reference)
"""Trainium2 Bass kernel for causal self-attention with RoPE.

Problem: B=2, T=2048, C=1024, H=16 heads, hd=64, fp32, causal, rotary embeddings.

Sharding: 8 cores = 2 batches x 4 head-groups. Core c handles batch c//4 and
heads [4*(c%4), 4*(c%4)+4). Each core computes its heads' Q/K/V projections,
RoPE, causal flash-style attention, and a partial output projection over its
256 input channels; the host sums the 4 partial projections per batch and adds
the output bias.

Device layout choices (per core):
  - x.T (channels-major, [1025, 2048], row 1024 = ones for the V bias) is
    pre-transposed on the host so every DMA is contiguous.
  - Q/K are produced channel-major ("QT"/"KT" = [128 ch, 2048 t]) with head
    channels packed [h_even(32) | h_odd(32)] per head, two heads per 128-row
    tile. RoPE is computed as QROT = (Qraw+bq)*CC + swap32((Qraw+bq)*SS) where
    CC/SS are host-precomputed cos/sin row patterns and swap32 exchanges the
    even/odd 32-row blocks via two small SBUF-SBUF DMAs.
  - Scores are computed transposed (S_T[s, t]) so the exp'd tile can feed the
    att@V matmul directly as the moving operand (no transposes anywhere).
  - Softmax denominators come from an appended ones-column on V (M=65), and
    division is deferred to a single per-head scale of the attention output.
  - Causality: s-tiles with s > t skipped at 128-col granularity, triangular
    diagonal sub-block masked with a -1e10 additive tile pre-exp.
"""

import os
import time
from contextlib import ExitStack

import ml_dtypes
import numpy as np

import concourse.bass as bass
import concourse.tile as tile
from concourse import bacc, library_config, mybir
from concourse.bass_utils import run_bass_kernel_spmd

F32 = mybir.dt.float32
# float32r runs the PE at 1 cycle/row (vs 4 for float32) when the moving free
# dim is >= 256. Tiles that feed matmuls are declared float32r end-to-end
# (the BIR verifier requires fp32r operands be *produced* as fp32r).
MM_DT = mybir.dt.float32r if os.environ.get("KERNEL_FP32R", "1") == "1" else F32
# attention (scores / att@V) operands in bf16: separate FWL weight loads and
# full-rate streaming; projections stay fp32r for accuracy
AT_DT = mybir.dt.bfloat16 if os.environ.get("ATTN_BF16", "1") == "1" else MM_DT
# projections in bf16 too: halves the input DMA and removes the fp32r
# fused-weight-load tax on the projection matmuls
PJ_DT = mybir.dt.bfloat16 if os.environ.get("PROJ_BF16", "1") == "1" else MM_DT

T = 2048
C = 1024
HD = 64
HPC = 4          # heads per core
NCORES = 8
NEG = -1e10

AF = mybir.ActivationFunctionType
ALU = mybir.AluOpType

LAST_EXEC_NS = None
LAST_RESULTS = None


def _mm(ap):
    return ap


def build_nc():
    nc = bacc.Bacc("TRN2", target_bir_lowering=False, debug=False)

    xT = nc.dram_tensor("xT", [C + 1, T], PJ_DT, kind="ExternalInput").ap()
    wqT = nc.dram_tensor("wqT", [C, 256], PJ_DT, kind="ExternalInput").ap()
    wkT = nc.dram_tensor("wkT", [C, 256], PJ_DT, kind="ExternalInput").ap()
    wvT = nc.dram_tensor("wvT", [C + 128, 256], PJ_DT, kind="ExternalInput").ap()
    wpT = nc.dram_tensor("wpT", [256, C], AT_DT, kind="ExternalInput").ap()
    bqk = nc.dram_tensor("bqk", [128, 4], F32, kind="ExternalInput").ap()
    cc_d = nc.dram_tensor("cc", [128, T], F32, kind="ExternalInput").ap()
    ss_d = nc.dram_tensor("ss", [128, T], F32, kind="ExternalInput").ap()
    tri_d = nc.dram_tensor("tri", [128, 128], F32, kind="ExternalInput").ap()
    out_d = nc.dram_tensor("out", [T, C], F32, kind="ExternalOutput").ap()

    with tile.TileContext(nc) as tc, ExitStack() as ctx:
        consts = ctx.enter_context(tc.tile_pool(name="consts", bufs=1))

        nc.gpsimd.load_library(library_config.attn)

        cc_sb = consts.tile([128, T], F32)
        ss_sb = consts.tile([128, T], F32)
        tri_sb = consts.tile([128, 128], F32)
        bqk_sb = consts.tile([128, 4], F32)

        # rotated Q^T / K^T, one [128, T] tile per head pair
        qkt = [consts.tile([128, T], AT_DT, name=f"qkt{i}") for i in range(4)]
        # V' tiles: per s-tile [128 s, 4*65] (64 v-cols + ones col per head)
        vp = [consts.tile([128, 4 * 65], AT_DT, name=f"vp{i}") for i in range(16)]

        # ---------------- phase 1: projections + rope ----------------
        with (
            tc.tile_pool(name="xpool", bufs=1) as xp,
            tc.tile_pool(name="wpool", bufs=1) as wpl,
            tc.tile_pool(name="qkpsum", bufs=2, space="PSUM") as qkp,
            tc.tile_pool(name="vpsum", bufs=2, space="PSUM") as vps_pool,
            tc.tile_pool(name="rope", bufs=2) as rp,
        ):
            xts = [xp.tile([128, T], PJ_DT, name=f"xt{j}") for j in range(8)]
            wq_sb = [wpl.tile([128, 256], PJ_DT, name=f"wq{j}") for j in range(8)]
            wk_sb = [wpl.tile([128, 256], PJ_DT, name=f"wk{j}") for j in range(8)]
            wv_sb = [wpl.tile([128, 256], PJ_DT, name=f"wv{j}") for j in range(9)]
            wv1 = wv_sb[8]
            h0, h1 = slice(0, 1024), slice(1024, 2048)
            # priority order: everything the first Q chunks need, j-interleaved
            # so the PE accumulation chain can start after j=0 arrives
            for j in range(8):
                nc.sync.dma_start(wq_sb[j][:], wqT[128 * j:128 * (j + 1), :])
                nc.sync.dma_start(xts[j][:, h0], xT[128 * j:128 * (j + 1), h0])
            nc.sync.dma_start(bqk_sb[:], bqk[:])
            nc.sync.dma_start(cc_sb[:, h0], cc_d[:, h0])
            nc.sync.dma_start(ss_sb[:, h0], ss_d[:, h0])
            nc.sync.dma_start(tri_sb[:], tri_d[:])
            for j in range(8):
                nc.sync.dma_start(wk_sb[j][:], wkT[128 * j:128 * (j + 1), :])
            for j in range(9):
                nc.sync.dma_start(wv_sb[j][:], wvT[128 * j:128 * (j + 1), :])
            for j in range(8):
                nc.sync.dma_start(xts[j][:, h1], xT[128 * j:128 * (j + 1), h1])
            nc.sync.dma_start(cc_sb[:, h1], cc_d[:, h1])
            nc.sync.dma_start(ss_sb[:, h1], ss_d[:, h1])

            # ones: x1 is fp32r (ACT constant producer); V ones cols are bf16
            # (memset is legal) on the idle gpsimd queue
            x1 = xp.tile([1, T], PJ_DT)
            if PJ_DT == mybir.dt.bfloat16:
                nc.gpsimd.memset(x1[:], 1.0)
            else:
                nc.scalar.activation(x1[:], cc_sb[0:1, :], AF.Copy,
                                     scale=0.0, bias=1.0)
            vview = [v.rearrange("p (h d) -> p h d", d=65) for v in vp]
            for tch in range(16):
                nc.gpsimd.memset(vview[tch][:, :, 64], 1.0)

            def qk_chunk(wsb, dst, i, half, bias):
                ps = qkp.tile([128, 1024], F32, tag="qkraw",
                              name=f"qkraw{dst.name}_{half}")
                for tg in range(2):
                    sl = slice(512 * tg, 512 * (tg + 1))
                    xsl = slice(1024 * half + 512 * tg,
                                1024 * half + 512 * tg + 512)
                    for j in range(8):
                        nc.tensor.matmul(
                            ps[:, sl],
                            _mm(wsb[j][:, 128 * i:128 * (i + 1)]),
                            _mm(xts[j][:, xsl]),
                            start=(j == 0), stop=(j == 7))
                hsl = slice(1024 * half, 1024 * (half + 1))
                p1 = rp.tile([128, 1024], F32, tag="p1")
                p2 = rp.tile([128, 1024], F32, tag="p2")
                p2s = rp.tile([128, 1024], F32, tag="p2s")
                nc.vector.scalar_tensor_tensor(
                    out=p1[:], in0=ps[:], scalar=bias,
                    in1=cc_sb[:, hsl], op0=ALU.add, op1=ALU.mult)
                nc.vector.scalar_tensor_tensor(
                    out=p2[:], in0=ps[:], scalar=bias,
                    in1=ss_sb[:, hsl], op0=ALU.add, op1=ALU.mult)
                for r in range(4):
                    src = slice(32 * (r ^ 1), 32 * (r ^ 1) + 32)
                    dstp = slice(32 * r, 32 * r + 32)
                    nc.sync.dma_start(p2s[dstp, :], p2[src, :])
                nc.vector.tensor_add(dst[:, hsl], p1[:], p2s[:])

            def v_chunk(tch):
                vraw = vps_pool.tile([128, 256], F32, tag="vraw",
                                     name=f"vraw{tch}")
                tsl = slice(128 * tch, 128 * (tch + 1))
                for j in range(8):
                    nc.tensor.matmul(
                        vraw[:], _mm(xts[j][:, tsl]), _mm(wv_sb[j][:]),
                        start=(j == 0), stop=False)
                nc.tensor.matmul(
                    vraw[:], _mm(x1[:, tsl]), _mm(wv1[0:1, :]),
                    start=False, stop=True)
                nc.vector.tensor_copy(vview[tch][:, :, 0:64], vraw[:])

            # half-0 work first (its loads arrive first), then half-1
            for i in range(2):
                qk_chunk(wq_sb, qkt[i], i, 0, bqk_sb[:, i:i + 1])
            for i in range(2):
                qk_chunk(wk_sb, qkt[2 + i], i, 0, bqk_sb[:, 2 + i:3 + i])
            for tch in range(8):
                v_chunk(tch)
            for i in range(2):
                qk_chunk(wq_sb, qkt[i], i, 1, bqk_sb[:, i:i + 1])
            for i in range(2):
                qk_chunk(wk_sb, qkt[2 + i], i, 1, bqk_sb[:, 2 + i:3 + i])
            for tch in range(8, 16):
                v_chunk(tch)

        # pools for phases 2+3 (opened after phase-1 pools release their SBUF)
        late = ctx.enter_context(tc.tile_pool(name="late", bufs=1))
        # scaled attention outputs (d-major), one [128, T] tile per head pair
        usc = [late.tile([128, T], AT_DT, name=f"usc{p}") for p in range(2)]
        # output projection weights
        wp_sb = [late.tile([128, C], AT_DT, name=f"wp{p}") for p in range(2)]
        for p in range(2):
            nc.sync.dma_start(wp_sb[p][:], wpT[128 * p:128 * (p + 1), :])

        # ---------------- phase 2: attention ----------------
        # 1024-wide score/exp windows (gp), head-serial so PSUM fits:
        # scores [128,1024] x2bufs (4 banks) + yz [65,512] x3 + op x1 = 8
        with (
            tc.tile_pool(name="spsum", bufs=2, space="PSUM") as sp,
            tc.tile_pool(name="oppsum", bufs=1, space="PSUM") as opp,
            tc.tile_pool(name="yzpsum", bufs=3, space="PSUM") as yzp,
            tc.tile_pool(name="epool", bufs=12) as epl,
            tc.tile_pool(name="rzpool", bufs=4) as rzp,
            tc.tile_pool(name="rzbpool", bufs=4) as rzbp,
            tc.tile_pool(name="ostage", bufs=4) as ost,
        ):
            for gp in range(2):
                ns = 8 * gp + 8
                for pr in range(2):
                    heads = (2 * pr, 2 * pr + 1)
                    kt_t, qt_t = qkt[2 + pr], qkt[pr]
                    for h in heads:
                        hs = h % 2
                        rows = slice(64 * hs, 64 * (hs + 1))
                        yzA = yzp.tile([65, 512], F32, tag="yz",
                                       name=f"yzA{gp}_{h}")
                        yzB = yzp.tile([65, 512], F32, tag="yz",
                                       name=f"yzB{gp}_{h}")
                        for i in range(ns):
                            sub0 = 128 * max(i - 8 * gp, 0)
                            s_ps = sp.tile([128, 1024], F32, tag="s",
                                           name=f"s{gp}_{i}_{h}")
                            for bk in range(2):
                                lo = max(sub0, 512 * bk)
                                hi = 512 * (bk + 1)
                                if lo >= hi:
                                    continue
                                nc.tensor.matmul(
                                    s_ps[:, lo:hi],
                                    _mm(kt_t[rows, 128 * i:128 * (i + 1)]),
                                    _mm(qt_t[rows,
                                             1024 * gp + lo:1024 * gp + hi]),
                                    start=True, stop=True)
                            if i >= 8 * gp:
                                dsl = slice(sub0, sub0 + 128)
                                nc.vector.tensor_add(
                                    s_ps[:, dsl], s_ps[:, dsl], tri_sb[:])
                            et = epl.tile([128, 1024], AT_DT, tag="e",
                                          name=f"e{gp}_{i}_{h}")
                            nc.scalar.activation(
                                et[:, sub0:], s_ps[:, sub0:],
                                AF.Exp, scale=0.125)
                            vsl = vp[i][:, 65 * h:65 * (h + 1)]
                            if sub0 < 512:
                                nc.tensor.matmul(
                                    yzA[:, sub0:512],
                                    _mm(vsl), _mm(et[:, sub0:512]),
                                    start=(i == 0), stop=(i == 8 * gp + 3))
                            sb = max(sub0, 512)
                            nc.tensor.matmul(
                                yzB[:, sb - 512:512],
                                _mm(vsl), _mm(et[:, sb:1024]),
                                start=(i == 0), stop=(i == ns - 1))
                        for half, yz in ((0, yzA), (1, yzB)):
                            g = 2 * gp + half
                            gsl = slice(512 * g, 512 * (g + 1))
                            zrow = rzp.tile([1, 512], F32, tag="zrow",
                                            name=f"zrow{g}_{h}")
                            nc.vector.tensor_copy(zrow[:], yz[64:65, :])
                            rzr = rzp.tile([1, 512], F32, tag="rzr",
                                           name=f"rzr{g}_{h}")
                            nc.vector.reciprocal_approx_fast(rzr[:], zrow[:])
                            rzb = rzbp.tile([64, 512], F32, tag="rzb",
                                            name=f"rzb{g}_{h}")
                            nc.gpsimd.partition_broadcast(rzb[:], rzr[:])
                            nc.vector.tensor_mul(
                                usc[pr][64 * hs:64 * (hs + 1), gsl],
                                yz[0:64, :], rzb[:])

                # output projection for this window's eight t-chunks.
                # gp=0 trickles through the dedicated 1-bank pool (it overlaps
                # gp=1's attention); gp=1 is the kernel tail, so it borrows
                # the by-then-idle 2-bank scores slots for 1024-wide units.
                for tch in range(8 * gp, 8 * gp + 8):
                    tsl = slice(128 * tch, 128 * (tch + 1))
                    if gp == 0:
                        for cg in range(2):
                            csl = slice(512 * cg, 512 * (cg + 1))
                            ps = opp.tile([128, 512], F32, tag="op",
                                          name=f"op{tch}_{cg}")
                            for pq in range(2):
                                nc.tensor.matmul(
                                    ps[:], _mm(usc[pq][:, tsl]),
                                    _mm(wp_sb[pq][:, csl]),
                                    start=(pq == 0), stop=(pq == 1))
                            st = ost.tile([128, 512], F32, tag="ost",
                                          name=f"ost{tch}_{cg}")
                            nc.vector.tensor_copy(st[:], ps[:])
                            nc.sync.dma_start(out_d[tsl, csl], st[:])
                    else:
                        ps = sp.tile([128, 1024], F32, tag="s",
                                     name=f"opw{tch}")
                        for cg in range(2):
                            csl = slice(512 * cg, 512 * (cg + 1))
                            for pq in range(2):
                                nc.tensor.matmul(
                                    ps[:, csl], _mm(usc[pq][:, tsl]),
                                    _mm(wp_sb[pq][:, csl]),
                                    start=(pq == 0), stop=(pq == 1))
                        st = ost.tile([128, 1024], F32, tag="ostw",
                                      name=f"ostw{tch}")
                        nc.vector.tensor_copy(st[:], ps[:])
                        nc.sync.dma_start(out_d[tsl, :], st[:])

    nc.compile()
    return nc


_NC_CACHE = {}


def _get_nc():
    if "nc" not in _NC_CACHE:
        _NC_CACHE["nc"] = build_nc()
    return _NC_CACHE["nc"]


def make_in_map(core, x, Wq, bq, Wk, bk, Wv, bv, Wp, bp, rope_cache):
    b = core // 4
    hbase = (core % 4) * 4

    pj_np = ml_dtypes.bfloat16 if os.environ.get("PROJ_BF16", "1") == "1" \
        else np.float32
    xTa = np.empty((C + 1, T), pj_np)
    xTa[:C] = np.asarray(x[b], np.float32).T
    xTa[C] = 1.0

    # packed channel order for Q/K: per pair p, heads (hbase+2p, hbase+2p+1),
    # rows [hA_even(32) | hA_odd(32) | hB_even(32) | hB_odd(32)]
    perm = []
    for p in range(2):
        for hh in range(2):
            h = hbase + 2 * p + hh
            perm += [h * HD + 2 * m for m in range(32)]
            perm += [h * HD + 2 * m + 1 for m in range(32)]
    perm = np.asarray(perm)

    wqTa = np.ascontiguousarray(np.asarray(Wq, np.float32)[perm, :].T).astype(pj_np)
    wkTa = np.ascontiguousarray(np.asarray(Wk, np.float32)[perm, :].T).astype(pj_np)

    chs = np.arange(hbase * HD, hbase * HD + 256)
    wvTa = np.zeros((C + 128, 256), pj_np)
    wvTa[:C] = np.asarray(Wv, np.float32)[chs, :].T
    wvTa[C] = np.asarray(bv, np.float32)[chs]
    wpTa = np.ascontiguousarray(
        np.asarray(Wp, np.float32)[:, chs].T).astype(ml_dtypes.bfloat16)

    bqp = np.asarray(bq, np.float32)[perm].reshape(2, 128).T
    bkp = np.asarray(bk, np.float32)[perm].reshape(2, 128).T
    bqk_a = np.concatenate([bqp, bkp], axis=1)  # [128, 4]

    rc = np.asarray(rope_cache, np.float32)  # [T, 32, 2]
    r = np.arange(128)
    m = r % 32
    sign = np.where((r % 64) < 32, 1.0, -1.0).astype(np.float32)
    cc_a = np.ascontiguousarray(rc[:, m, 0].T)            # [128, T]
    ss_a = np.ascontiguousarray(rc[:, m, 1].T * sign[:, None])

    sl, tl = np.arange(128)[:, None], np.arange(128)[None, :]
    tri_a = np.where(tl >= sl, 0.0, NEG).astype(np.float32)

    return dict(xT=xTa, wqT=wqTa, wkT=wkTa, wvT=wvTa, wpT=wpTa,
                bqk=bqk_a, cc=cc_a, ss=ss_a, tri=tri_a)


def kernel(x, Wq, bq, Wk, bk, Wv, bv, Wp, bp, rope_cache):
    global LAST_EXEC_NS, LAST_RESULTS
    args = (x, Wq, bq, Wk, bk, Wv, bv, Wp, bp, rope_cache)
    nc = _get_nc()
    in_maps = [make_in_map(c, *args) for c in range(NCORES)]
    r = None
    for attempt in range(4):
        try:
            r = run_bass_kernel_spmd(nc, in_maps, list(range(NCORES)))
            break
        except Exception:
            # transient NRT exec-unit errors recover on re-dispatch
            if attempt == 3:
                raise
            time.sleep(5.0 * (attempt + 1))
    LAST_EXEC_NS = r.exec_time_ns
    LAST_RESULTS = r
    out = np.zeros((2, T, C), np.float32)
    for core in range(NCORES):
        out[core // 4] += r.results[core]["out"]
    out += np.asarray(bp, np.float32)[None, None, :]
    return out

